# revision 15
# baseline (speedup 1.0000x reference)
"""Trainium2 Bass kernel: single attention head (B=8, S=2048, E=1024, H=64).

Sharding: data-parallel over batch -- each of the 8 NeuronCores computes one
batch element's full attention. No collectives; every HBM byte read once.

v3: column-blocked streaming pipeline, dense PE schedule.
  - Inputs staged host-side as fp16 quarter-slabs [NB, 128, EC*BW]:
    slab[q][p][c*BW+s] = x[q*BW+s, c*128+p].  Each input DMA is a flat 2D
    pattern (8KB contiguous per partition) -> cheap HWDGE trigger instead of
    ~2.9us for a 3-level pattern.
  - One DMA priority stream on the sync ring, ordered so the last-arriving
    tensor gates the least dependent work.
  - Scores computed transposed per (block, key-tile-pair): S^T[sk,sq] =
    kt.T @ qt into PSUM pair-tiles [128,2,512]; one ScalarE exp instruction
    covers 1024 columns.  ScalarE exp (~34us) overlaps everything.
  - The PE stream is kept DENSE (qproj/vproj matmuls, AV pairs, and
    finalize transposes spread between score pairs) so the PE p-state ramps
    to 2.4GHz and stays there.
  - AV pairs (ones-augmented v gives softmax denominators for free) are
    scheduled per vaug-quarter availability; per-block finalize
    (transpose/normalize/store) rides along -> small tail.
  - v^T tiles produced by HWDGE transpose-DMA triggered from the scalar
    ring (positioned between exps so they never head-of-line stall it).
  - PSUM: scores 2x2 banks + proj 1 + oa 2 + fin-transpose 1 = 8 exactly.
"""

import numpy as np

import concourse.bass as bass  # noqa: F401  (engine namespaces live on nc)
import concourse.mybir as mybir
import concourse.tile as tile
from concourse import bacc
from concourse.bass_utils import run_bass_kernel_spmd
from concourse.masks import make_identity

B, S, E, H = 8, 2048, 1024, 64
EC = E // 128   # contraction chunks per projection
NT = S // 128   # key tiles
NB = 4          # 512-column blocks
BW = S // NB
F16 = mybir.dt.float16
F32 = mybir.dt.float32

_CACHE = {}


def _build_nc():
    nc = bacc.Bacc(None)
    xq = nc.declare_dram_parameter("xqs", [NB, 128, EC * BW], F16, isOutput=False)
    xk = nc.declare_dram_parameter("xks", [NB, 128, EC * BW], F16, isOutput=False)
    xv = nc.declare_dram_parameter("xvs", [NB, 128, EC * BW], F16, isOutput=False)
    wq = nc.declare_dram_parameter("wq", [E, H], F16, isOutput=False)
    wk = nc.declare_dram_parameter("wk", [E, H], F16, isOutput=False)
    wv = nc.declare_dram_parameter("wv", [E, H], F16, isOutput=False)
    bq = nc.declare_dram_parameter("bq", [H, 1], F32, isOutput=False)
    bv = nc.declare_dram_parameter("bv", [H, 1], F32, isOutput=False)
    out = nc.declare_dram_parameter("out", [S, H], F32, isOutput=True)

    Exp = mybir.ActivationFunctionType.Exp

    with tile.TileContext(nc) as tc:
        with tc.tile_pool(name="const", bufs=1) as const, \
             tc.tile_pool(name="xqp", bufs=4) as xqp, \
             tc.tile_pool(name="xvp", bufs=4) as xvp, \
             tc.tile_pool(name="oassb", bufs=2) as oassb, \
             tc.tile_pool(name="osbp", bufs=2) as osbp, \
             tc.tile_pool(name="rcp", bufs=2) as rcp:

            # -- small constants at the head of the gpsimd ring --
            wts = {}
            for nm, dram in (("k", wk), ("q", wq), ("v", wv)):
                wt = const.tile([128, EC, H], F16, name=f"w{nm}")
                nc.gpsimd.dma_start(
                    out=wt[:], in_=dram[:].rearrange("(c p) h -> p c h", p=128))
                wts[nm] = wt
            bq_t = const.tile([H, 1], F32, name="bq_t")
            nc.gpsimd.dma_start(out=bq_t[:], in_=bq[:])
            bv_t = const.tile([H, 1], F32, name="bv_t")
            nc.gpsimd.dma_start(out=bv_t[:], in_=bv[:])

            kt = const.tile([64, S], F16, name="kt")
            qt = const.tile([64, S], F16, name="qt")
            vt = const.tile([64, S], F16, name="vt")
            vaug = const.tile([128, NT, 80], F16, name="vaug")
            ident = const.tile([128, 128], F16, name="ident")
            warm = const.tile([1, 8], F16, name="warm")

            nc.gpsimd.memset(vaug[:, :, 64], 1.0)
            make_identity(nc, ident[:])

            # -- big input DMAs: sync ring, explicit priority order --
            xq_b, xv_q = [], []
            for t in range(NB):
                xq_b.append(xqp.tile([128, EC, BW], F16, tag="xq", name=f"xq{t}"))
                xv_q.append(xvp.tile([128, EC, BW], F16, tag="xv", name=f"xv{t}"))

            def dma_slab(dst, dram, q):
                nc.sync.dma_start(
                    out=dst[:], in_=dram[q].rearrange("p (c s) -> p c s", c=EC))

            # xk pool lives only through the k projection; closing it early
            # lets ptp (4 bufs) reuse its SBUF
            with tc.tile_pool(name="xkp", bufs=4) as xkp:
                xk_q = []
                for t in range(NB):
                    xk_q.append(
                        xkp.tile([128, EC, BW], F16, tag="xk", name=f"xk{t}"))

                dma_slab(xk_q[0], xk, 0)
                dma_slab(xq_b[0], xq, 0)
                dma_slab(xk_q[1], xk, 1)
                dma_slab(xk_q[2], xk, 2)
                dma_slab(xq_b[1], xq, 1)
                dma_slab(xk_q[3], xk, 3)
                dma_slab(xv_q[0], xv, 0)
                dma_slab(xq_b[2], xq, 2)
                dma_slab(xv_q[1], xv, 1)
                dma_slab(xq_b[3], xq, 3)
                dma_slab(xv_q[2], xv, 2)
                dma_slab(xv_q[3], xv, 3)

                # warm the Exp activation table off the critical path
                nc.scalar.activation(warm[:], ident[0:1, 0:8], Exp, scale=0.125)

                # -- k projection, quarter by quarter as DMA lands --
                with tc.tile_pool(name="kpp", bufs=1, space="PSUM") as kpp:
                    psk = kpp.tile([64, S], F32, name="psk")
                    for t in range(NB):
                        for c in range(EC):
                            nc.tensor.matmul(
                                psk[:, t * BW:(t + 1) * BW],
                                wts["k"][:, c, :], xk_q[t][:, c, :],
                                start=(c == 0), stop=(c == EC - 1))
                        nc.vector.tensor_copy(
                            kt[:, t * BW:(t + 1) * BW],
                            psk[:, t * BW:(t + 1) * BW])

            # -- main pipeline --
            with tc.tile_pool(name="ptp", bufs=4) as ptp, \
                 tc.tile_pool(name="sps", bufs=2, space="PSUM") as sps, \
                 tc.tile_pool(name="pps", bufs=1, space="PSUM") as pps, \
                 tc.tile_pool(name="oap", bufs=2, space="PSUM") as oap, \
                 tc.tile_pool(name="trp", bufs=1, space="PSUM") as trp:

                pts = [None] * NB
                oas = [None] * NB
                prj = [None]

                def proj_mm(w, xtile, dst, dsti, c):
                    # one filler matmul of a q/v projection (shared psum buf)
                    if c == 0:
                        prj[0] = pps.tile([64, BW], F32, tag="pp", name=f"pp{w}{dsti}")
                    nc.tensor.matmul(
                        prj[0][:], wts[w][:, c, :], xtile[:, c, :],
                        start=(c == 0), stop=(c == EC - 1),
                        skip_group_check=True)
                    if c == EC - 1:
                        bias = bq_t if w == "q" else bv_t
                        tgt = qt if w == "q" else vt
                        nc.vector.tensor_scalar_add(
                            tgt[:, dsti * BW:(dsti + 1) * BW], prj[0][:], bias[:])

                def qproj_mm(j, c):
                    proj_mm("q", xq_b[j], qt, j, c)

                def vproj_mm(q, c):
                    proj_mm("v", xv_q[q], vt, q, c)

                def vaug_t(q):
                    # HWDGE transpose trigger on the scalar ring
                    nc.scalar.dma_start_transpose(
                        vaug[:, 4 * q:4 * (q + 1), 0:64],
                        vt[:, q * BW:(q + 1) * BW])

                def av_pair(j, t2):
                    for t in (t2, t2 + 1):
                        nc.tensor.matmul(
                            oas[j][:], vaug[:, t, 0:65], pts[j][:, t, :],
                            start=(t == 0), stop=(t == NT - 1),
                            skip_group_check=True)

                def scores_pair(j, i):
                    st = sps.tile([128, 2, BW], F32, tag="st", name=f"st{j}_{i}")
                    for u in range(2):
                        nc.tensor.matmul(
                            st[:, u, :],
                            kt[:, (2 * i + u) * 128:(2 * i + u + 1) * 128],
                            qt[:, j * BW:(j + 1) * BW],
                            start=True, stop=True)
                    nc.scalar.activation(
                        pts[j][:, 2 * i:2 * i + 2, :], st[:], Exp, scale=0.125)

                def fin(j):
                    oasb = oassb.tile([65, BW], F16, tag="oasb", name=f"oasb{j}")
                    nc.vector.tensor_copy(oasb[:], oas[j][:])
                    osb = osbp.tile([128, 4, H], F32, tag="osb", name=f"osb{j}")
                    for jj in range(4):
                        tr = trp.tile([128, 65], F16, tag="tr", name=f"tr{j}_{jj}")
                        nc.tensor.transpose(
                            tr[:], oasb[:, jj * 128:(jj + 1) * 128],
                            ident[0:65, 0:65])
                        rc = rcp.tile([128, 1], F32, tag="rc", name=f"rc{j}_{jj}")
                        nc.vector.reciprocal(rc[:], tr[:, 64:65])
                        nc.vector.tensor_scalar(
                            osb[:, jj, :], tr[:, 0:64], rc[:], None,
                            op0=mybir.AluOpType.mult)
                    out_r = out[:].rearrange("(t p) h -> p t h", p=128)
                    nc.gpsimd.dma_start(
                        out=out_r[:, 4 * j:4 * (j + 1), :], in_=osb[:])

                # q projection for block 0 (start the exp stream asap)
                pts[0] = ptp.tile([128, NT, BW], F16, tag="pt", name="pt0")
                for c in range(EC):
                    qproj_mm(0, c)

                # block 0 scores; fillers: block-1 q projection
                for i in range(NT // 2):
                    scores_pair(0, i)
                    if i >= 4:
                        qproj_mm(1, 2 * (i - 4))
                        qproj_mm(1, 2 * (i - 4) + 1)

                # block 1: fillers = vproj 0, qproj 2, first AV(0) pairs
                pts[1] = ptp.tile([128, NT, BW], F16, tag="pt", name="pt1")
                oas[0] = oap.tile([65, BW], F32, tag="oa", name="oa0")
                for i in range(NT // 2):
                    scores_pair(1, i)
                    if i in (1, 2):
                        for c in range(4 * (i - 1), 4 * (i - 1) + 4):
                            vproj_mm(0, c)
                    if i == 4:
                        vaug_t(0)
                    if 3 <= i <= 6:
                        qproj_mm(2, 2 * (i - 3))
                        qproj_mm(2, 2 * (i - 3) + 1)
                    if i == 6:
                        av_pair(0, 0)
                    if i == 7:
                        av_pair(0, 2)

                # block 2: fillers = vproj 1, qproj 3, AV(0) mid, AV(1) start
                pts[2] = ptp.tile([128, NT, BW], F16, tag="pt", name="pt2")
                oas[1] = oap.tile([65, BW], F32, tag="oa", name="oa1")
                for i in range(NT // 2):
                    scores_pair(2, i)
                    if i in (0, 1):
                        for c in range(4 * i, 4 * i + 4):
                            vproj_mm(1, c)
                    if i == 1:
                        vaug_t(1)
                    if i in (2, 3):
                        av_pair(0, 4 + 2 * (i - 2))
                    if 2 <= i <= 5:
                        qproj_mm(3, 2 * (i - 2))
                        qproj_mm(3, 2 * (i - 2) + 1)
                    if i in (4, 5):
                        av_pair(1, 2 * (i - 4))
                    if i in (6, 7):
                        av_pair(1, 4 + 2 * (i - 6))

                # block 3: fillers = vproj 2,3, AV(1) mid, fin(0), AV(2) start
                pts[3] = ptp.tile([128, NT, BW], F16, tag="pt", name="pt3")
                for i in range(NT // 2):
                    scores_pair(3, i)
                    if i in (0, 1):
                        for c in range(4 * i, 4 * i + 4):
                            vproj_mm(2, c)
                    if i == 1:
                        vaug_t(2)
                    if i in (2, 3):
                        for c in range(4 * (i - 2), 4 * (i - 2) + 4):
                            vproj_mm(3, c)
                        av_pair(0, 8 + 2 * (i - 2))
                    if i == 4:
                        vaug_t(3)
                    if i in (4, 5):
                        av_pair(1, 8 + 2 * (i - 4))
                    if i in (6, 7):
                        av_pair(0, 12 + 2 * (i - 6))

                # tail
                av_pair(1, 12)
                av_pair(1, 14)
                fin(0)
                oas[2] = oap.tile([65, BW], F32, tag="oa", name="oa2")
                for p in range(8):
                    av_pair(2, 2 * p)
                fin(1)
                oas[3] = oap.tile([65, BW], F32, tag="oa", name="oa3")
                for p in range(8):
                    av_pair(3, 2 * p)
                fin(2)
                fin(3)

    nc.finalize()
    return nc


def get_nc():
    if "nc" not in _CACHE:
        _CACHE["nc"] = _build_nc()
    return _CACHE["nc"]


def _slab(x):
    # [S, E] f32 -> [NB, 128, EC*BW] f16, slab[q, p, c*BW+s] = x[q*BW+s, c*128+p]
    a = x.reshape(NB, BW, EC, 128).transpose(0, 3, 2, 1).astype(np.float16)
    return np.ascontiguousarray(a.reshape(NB, 128, EC * BW))


def make_in_maps(inputs):
    q = np.asarray(inputs["query"], np.float32)
    k = np.asarray(inputs["key_"], np.float32)
    v = np.asarray(inputs["value"], np.float32)
    wq = np.ascontiguousarray(np.asarray(inputs["Wq"], np.float32).astype(np.float16))
    wk = np.ascontiguousarray(np.asarray(inputs["Wk"], np.float32).astype(np.float16))
    wv = np.ascontiguousarray(np.asarray(inputs["Wv"], np.float32).astype(np.float16))
    bq = np.ascontiguousarray(np.asarray(inputs["bq"], np.float32).reshape(H, 1))
    bv = np.ascontiguousarray(np.asarray(inputs["bv"], np.float32).reshape(H, 1))
    in_maps = []
    for b in range(B):
        in_maps.append({
            "xqs": _slab(q[b]),
            "xks": _slab(k[b]),
            "xvs": _slab(v[b]),
            "wq": wq, "wk": wk, "wv": wv,
            "bq": bq, "bv": bv,
        })
    return in_maps


def kernel(**inputs):
    nc = get_nc()
    in_maps = make_in_maps(inputs)
    res = run_bass_kernel_spmd(nc, in_maps, list(range(B)))
    return np.stack([res.results[b]["out"] for b in range(B)], axis=0)
